# revision 1
# baseline (speedup 1.0000x reference)
"""GaborAutoencoder forward pass, data-parallel across 8 NeuronCores.

Strategy (per sharding_hint): pure data parallel — shard the batch dim of x
across the 8 cores, replicate the small MLP weights on every core. Each core
runs the encoder MLP + Gabor wavelet synthesis for its 512-row shard; the
full (4096, 2, 2048) output is the concatenation of shards (no collectives
needed in forward).
"""
import numpy as np
import jax
import jax.numpy as jnp
from jax.sharding import Mesh, PartitionSpec as P
from jax.experimental.shard_map import shard_map
from functools import partial

SIGNAL_LEN = 2048
N_WAVELETS = 32
TWO_PI = 2.0 * np.pi
N_CORES = 8

_compiled = {}


def _forward_shard(x, W1, b1, W2, b2, W3, b3, W4, b4):
    # x: (B_shard, 2, 2048) on one core
    B = x.shape[0]
    h = x.reshape(B, -1)
    h = jax.nn.relu(h @ W1.T + b1)
    h = jax.nn.relu(h @ W2.T + b2)
    h = jax.nn.relu(h @ W3.T + b3)
    p = (h @ W4.T + b4).reshape(B, N_WAVELETS, 5)

    A = p[..., 0]
    t0 = jax.nn.sigmoid(p[..., 1]) * SIGNAL_LEN
    f = jax.nn.sigmoid(p[..., 2]) * 0.5
    sigma = jax.nn.sigmoid(p[..., 3]) * 200.0 + 2.0
    phi = p[..., 4]

    t = jnp.arange(SIGNAL_LEN, dtype=x.dtype)
    sig = jnp.zeros((B, SIGNAL_LEN), x.dtype)
    # static unroll over the 32 wavelets keeps peak memory at O(B*T)
    for n in range(N_WAVELETS):
        dt = t[None, :] - t0[:, n, None]
        g = (A[:, n, None]
             * jnp.exp(-(dt * dt) / (2.0 * sigma[:, n, None] ** 2))
             * jnp.cos(TWO_PI * f[:, n, None] * dt + phi[:, n, None]))
        sig = sig + g
    return jnp.stack([sig, sig], axis=1)  # (B_shard, 2, 2048)


def _get_compiled():
    if "fn" in _compiled:
        return _compiled["fn"]
    devices = jax.devices()[:N_CORES]
    mesh = Mesh(np.asarray(devices), ("b",))
    in_specs = (P("b"),) + (P(),) * 8
    fn = jax.jit(
        shard_map(
            _forward_shard,
            mesh=mesh,
            in_specs=in_specs,
            out_specs=P("b"),
            check_rep=False,
        )
    )
    _compiled["fn"] = fn
    _compiled["mesh"] = mesh
    return fn


def kernel(x, W1, b1, W2, b2, W3, b3, W4, b4):
    fn = _get_compiled()
    args = [np.asarray(x, np.float32)] + [
        np.asarray(a, np.float32) for a in (W1, b1, W2, b2, W3, b3, W4, b4)
    ]
    out = fn(*args)
    return np.asarray(out).astype(np.float32)


# revision 2
# speedup vs baseline: 1.4486x; 1.4486x over previous
"""GaborAutoencoder forward pass, data-parallel across 8 NeuronCores.

Strategy (per sharding_hint): pure data parallel — shard the batch dim of x
across the 8 cores, replicate the small MLP weights on every core. Each core
runs the encoder MLP + Gabor wavelet synthesis for its 512-row shard; the
full (4096, 2, 2048) output is the concatenation of shards (no collectives
needed in forward).
"""
import numpy as np
import jax
import jax.numpy as jnp
from jax.sharding import Mesh, PartitionSpec as P
from jax.experimental.shard_map import shard_map
from functools import partial

SIGNAL_LEN = 2048
N_WAVELETS = 32
TWO_PI = 2.0 * np.pi
N_CORES = 8

_compiled = {}


def _forward_shard(x, W1, b1, W2, b2, W3, b3, W4, b4):
    # x: (B_shard, 2, 2048) on one core
    B = x.shape[0]
    h = x.reshape(B, -1)
    h = jax.nn.relu(h @ W1.T + b1)
    h = jax.nn.relu(h @ W2.T + b2)
    h = jax.nn.relu(h @ W3.T + b3)
    p = (h @ W4.T + b4).reshape(B, N_WAVELETS, 5)

    A = p[..., 0]
    t0 = jax.nn.sigmoid(p[..., 1]) * SIGNAL_LEN
    f = jax.nn.sigmoid(p[..., 2]) * 0.5
    sigma = jax.nn.sigmoid(p[..., 3]) * 200.0 + 2.0
    phi = p[..., 4]

    # chunk the time axis: (B, N, 512) intermediates keep memory bounded and
    # fuse better than 32 per-wavelet passes
    CH = 512
    outs = []
    for c in range(SIGNAL_LEN // CH):
        t = jnp.arange(c * CH, (c + 1) * CH, dtype=x.dtype)
        dt = t[None, None, :] - t0[..., None]
        g = (A[..., None]
             * jnp.exp(-(dt * dt) / (2.0 * sigma[..., None] ** 2))
             * jnp.cos(TWO_PI * f[..., None] * dt + phi[..., None]))
        outs.append(g.sum(axis=1))
    sig = jnp.concatenate(outs, axis=1)
    return jnp.stack([sig, sig], axis=1)  # (B_shard, 2, 2048)


def _get_compiled():
    if "fn" in _compiled:
        return _compiled["fn"]
    devices = jax.devices()[:N_CORES]
    mesh = Mesh(np.asarray(devices), ("b",))
    in_specs = (P("b"),) + (P(),) * 8
    fn = jax.jit(
        shard_map(
            _forward_shard,
            mesh=mesh,
            in_specs=in_specs,
            out_specs=P("b"),
            check_rep=False,
        )
    )
    _compiled["fn"] = fn
    _compiled["mesh"] = mesh
    return fn


def kernel(x, W1, b1, W2, b2, W3, b3, W4, b4):
    fn = _get_compiled()
    args = [np.asarray(x, np.float32)] + [
        np.asarray(a, np.float32) for a in (W1, b1, W2, b2, W3, b3, W4, b4)
    ]
    out = fn(*args)
    return np.asarray(out).astype(np.float32)
